# revision 1
# baseline (speedup 1.0000x reference)
"""Trainium2 Bass kernel for nn_HCMGNNBasedMetaPathModel.

Strategy: bacteria nodes sharded over 8 cores (3750 -> padded 3840 rows each);
trait side + weights replicated. Edge-list segment ops are reformulated as
dense (deg-normalized) adjacency matmuls built on the host in bf16.
Per layer one 2MB AllReduce combines the trait-side partial aggregates.
On-device layout is node-major [128-row tiles, 256 free]; feature
contractions take a d-major (PE-transposed) bf16 copy as the stationary
operand. Linear-algebra folds remove all other transposes:
  tb@Wr.T = xb@(Wr@Wt).T + (Wr@bt)      (lin_r branch)
  ttl     = xt@(Wl_b@Wt_t).T + Wl_b@bt_t (pre-multiplied neighbor features)
  mean scaling folded into adjacency rows (host), counts exact there.
"""
import contextlib
import sys

for _p in ("/opt/trn_rl_repo",):
    if _p not in sys.path:
        sys.path.insert(0, _p)

import numpy as np
import ml_dtypes

import concourse.bass as bass
import concourse.tile as tile
from concourse import bacc, mybir
from concourse.bass_utils import run_bass_kernel_spmd

BF16 = ml_dtypes.bfloat16
F32 = mybir.dt.float32
BF = mybir.dt.bfloat16
AF = mybir.ActivationFunctionType
ALU = mybir.AluOpType
ts, ds = bass.ts, bass.ds

N_B, N_T, D, L, M = 30000, 2000, 256, 3, 2
NC = 8
B_SH = 3750          # real bacteria rows per core
BP = 3840            # padded bacteria rows per core
NBT = BP // 128      # 30 node tiles
TP = 2048            # padded trait rows
NTT = TP // 128      # 16 trait tiles
LN_EPS = 1e-5

# ---------------------------------------------------------------------------
# Host-side preprocessing
# ---------------------------------------------------------------------------

def _dense_adj(src, dst, n_dst, n_src):
    """A[d, s] = #edges (s->d), rows scaled by 1/max(deg,1)."""
    idx = dst.astype(np.int64) * n_src + src.astype(np.int64)
    A = np.bincount(idx, minlength=n_dst * n_src).astype(np.float32)
    A = A.reshape(n_dst, n_src)
    deg = np.bincount(dst.astype(np.int64), minlength=n_dst).astype(np.float32)
    A *= (1.0 / np.maximum(deg, 1.0))[:, None]
    return A


def _prep(inp):
    f32 = np.float32
    emb_b = np.asarray(inp["emb_b"], f32)
    emb_t = np.asarray(inp["emb_t"], f32)

    A_tb = _dense_adj(np.asarray(inp["src_tb"]), np.asarray(inp["dst_tb"]), N_B, N_T)
    A_bt = _dense_adj(np.asarray(inp["src_bt"]), np.asarray(inp["dst_bt"]), N_T, N_B)
    mpw = np.asarray(inp["mp_w"], np.float64)
    e = np.exp(mpw - mpw.max())
    w = e / e.sum()
    sw = float(w.sum())
    mp_adj = np.asarray(inp["mp_adj"], f32)
    A_mp = (w[0] * mp_adj[0].astype(np.float64) +
            w[1] * mp_adj[1].astype(np.float64)).astype(f32)

    xb0 = np.zeros((NC, NBT, 128, D), f32)
    xb0.reshape(NC, BP, D)[:, :B_SH] = emb_b.reshape(NC, B_SH, D)
    xt0 = np.zeros((NTT, 128, D), f32)
    xt0.reshape(TP, D)[:N_T] = emb_t

    def shard_T(A):  # [N_B, N_T] -> per-core [NTT, 128, BP] trait-major bf16
        out = np.zeros((NC, NTT, 128, BP), BF16)
        for c in range(NC):
            blk = np.zeros((TP, BP), f32)
            blk[:N_T, :B_SH] = A[c * B_SH:(c + 1) * B_SH].T
            out[c] = blk.reshape(NTT, 128, BP).astype(BF16)
        return out

    At = shard_T(A_tb)
    Amp = shard_T(A_mp)
    Abt = np.zeros((NC, NBT, 128, TP), BF16)
    for c in range(NC):
        blk = np.zeros((BP, TP), f32)
        blk[:B_SH, :N_T] = A_bt[:, c * B_SH:(c + 1) * B_SH].T
        Abt[c] = blk.reshape(NBT, 128, TP).astype(BF16)

    Wt_b, bt_b = np.asarray(inp["Wt_b"], f32), np.asarray(inp["bt_b"], f32)
    Wt_t, bt_t = np.asarray(inp["Wt_t"], f32), np.asarray(inp["bt_t"], f32)
    Wl_b, bl_b = np.asarray(inp["Wl_b"], f32), np.asarray(inp["bl_b"], f32)
    Wr_b = np.asarray(inp["Wr_b"], f32)
    Wl_t, bl_t = np.asarray(inp["Wl_t"], f32), np.asarray(inp["bl_t"], f32)
    Wr_t = np.asarray(inp["Wr_t"], f32)

    wlist, wi = [], {}

    def addW(name, WT):
        wi[name] = len(wlist)
        for kc in range(WT.shape[0] // 128):
            wlist.append(np.ascontiguousarray(WT[kc * 128:(kc + 1) * 128]).astype(BF16))

    for i in range(L):
        addW(("WtT_b", i), Wt_b[i].T)
        addW(("WcT_b", i), (Wr_b[i] @ Wt_b[i]).T)
        addW(("WctT", i), (Wl_b[i] @ Wt_t[i]).T)
        addW(("WtT_t", i), Wt_t[i].T)
        addW(("WcT_t", i), (Wr_t[i] @ Wt_t[i]).T)
        addW(("WlT_t", i), Wl_t[i].T)
    mpW = np.asarray(inp["mpW"], f32)
    Wp1b = np.asarray(inp["Wp1b"], f32)
    Wp1t = np.asarray(inp["Wp1t"], f32)
    addW("mpWT", mpW.T)
    addW("mpWT_x", sw * mpW.T)
    addW("Wp1baT", Wp1b[:, :D].T)
    addW("Wp1bbT", Wp1b[:, D:].T)
    addW("Wp1tT", Wp1t.T)
    Wc256 = np.stack(wlist)

    w128list, w128i = [], {}

    def addW128(name, WT):
        w128i[name] = len(w128list)
        for kc in range(WT.shape[0] // 128):
            w128list.append(np.ascontiguousarray(WT[kc * 128:(kc + 1) * 128]).astype(BF16))

    addW128("Wp2bT", np.asarray(inp["Wp2b"], f32).T)
    addW128("Wp2tT", np.asarray(inp["Wp2t"], f32).T)
    W128 = np.stack(w128list)

    vlist, vi = [], {}

    def addV(name, v):
        vi[name] = len(vlist)
        vlist.append(np.ascontiguousarray(
            np.broadcast_to(v.astype(f32), (128, v.shape[0]))).astype(BF16))

    for i in range(L):
        addV(("bt_b", i), bt_b[i])
        addV(("blc_b", i), bl_b[i] + Wr_b[i] @ bt_b[i])
        addV(("vttl", i), Wl_b[i] @ bt_t[i])
        addV(("lng_b", i), np.asarray(inp["lng_b"], f32)[i])
        addV(("lnb_b", i), np.asarray(inp["lnb_b"], f32)[i])
        addV(("bt_t", i), bt_t[i])
        addV(("blc_t", i), bl_t[i] + Wr_t[i] @ bt_t[i])
        addV(("lng_t", i), np.asarray(inp["lng_t"], f32)[i])
        addV(("lnb_t", i), np.asarray(inp["lnb_t"], f32)[i])
    for nm in ("mpb", "mplng", "mplnb", "bp1b", "plngb", "plnbb",
               "bp1t", "plngt", "plnbt"):
        addV(nm, np.asarray(inp[nm], f32))
    V256 = np.stack(vlist)

    v128list, v128i = [], {}
    for nm in ("bp2b", "bp2t"):
        v128i[nm] = len(v128list)
        v128list.append(np.ascontiguousarray(
            np.broadcast_to(np.asarray(inp[nm], f32), (128, 128))).astype(BF16))
    V128 = np.stack(v128list)

    ident = np.eye(128, dtype=f32)
    temp = float(np.asarray(inp["temperature"]).reshape(-1)[0])
    simb = float(np.asarray(inp["sim_bias"]).reshape(-1)[0])

    shared = dict(xt0=xt0, Wc256=Wc256, W128=W128, V256=V256, V128=V128,
                  ident=ident)
    in_maps = []
    for c in range(NC):
        m = dict(shared)
        m["xb0"] = np.ascontiguousarray(xb0[c])
        m["At"] = np.ascontiguousarray(At[c])
        m["Abt"] = np.ascontiguousarray(Abt[c])
        m["Amp"] = np.ascontiguousarray(Amp[c])
        in_maps.append(m)
    meta = dict(wi=wi, w128i=w128i, vi=vi, v128i=v128i,
                wcount=len(wlist), vcount=len(vlist), temp=temp, simb=simb)
    return in_maps, meta


# ---------------------------------------------------------------------------
# Device program
# ---------------------------------------------------------------------------

def build_program(meta):
    nc = bacc.Bacc("TRN2", target_bir_lowering=False, debug=False,
                   num_devices=NC)
    wi, w128i, vi, v128i = meta["wi"], meta["w128i"], meta["vi"], meta["v128i"]
    NW, NV = meta["wcount"], meta["vcount"]
    temp = meta["temp"]

    xb0_d = nc.dram_tensor("xb0", [NBT, 128, D], F32, kind="ExternalInput")
    xt0_d = nc.dram_tensor("xt0", [NTT, 128, D], F32, kind="ExternalInput")
    At_d = nc.dram_tensor("At", [NTT, 128, BP], BF, kind="ExternalInput")
    Abt_d = nc.dram_tensor("Abt", [NBT, 128, TP], BF, kind="ExternalInput")
    Amp_d = nc.dram_tensor("Amp", [NTT, 128, BP], BF, kind="ExternalInput")
    Wc_d = nc.dram_tensor("Wc256", [NW, 128, D], BF, kind="ExternalInput")
    W128_d = nc.dram_tensor("W128", [4, 128, 128], BF, kind="ExternalInput")
    V256_d = nc.dram_tensor("V256", [NV, 128, D], BF, kind="ExternalInput")
    V128_d = nc.dram_tensor("V128", [2, 128, 128], BF, kind="ExternalInput")
    id_d = nc.dram_tensor("ident", [128, 128], F32, kind="ExternalInput")
    sim_d = nc.dram_tensor("simO", [NBT, 128, TP], F32, kind="ExternalOutput")

    with tile.TileContext(nc) as tc, contextlib.ExitStack() as ctx:
        cpool = ctx.enter_context(tc.tile_pool(name="const", bufs=1))
        fpool = ctx.enter_context(tc.tile_pool(name="feat", bufs=1))
        spool = ctx.enter_context(tc.tile_pool(name="stream", bufs=4))
        bpool = ctx.enter_context(tc.tile_pool(name="abt_stream", bufs=3))
        epool = ctx.enter_context(tc.tile_pool(name="epi", bufs=3))
        qpool = ctx.enter_context(tc.tile_pool(name="sq", bufs=3))
        tpool = ctx.enter_context(tc.tile_pool(name="tiny", bufs=8))
        ppool = ctx.enter_context(tc.tile_pool(name="pscr", bufs=1))
        dpool = ctx.enter_context(tc.tile_pool(name="dram", bufs=2, space="DRAM"))

        # ---- constants ----
        wc = cpool.tile([128, NW, D], BF)
        for j in range(NW):
            nc.sync.dma_start(wc[:, j, :], Wc_d[j])
        w128 = cpool.tile([128, 4, 128], BF)
        for j in range(4):
            nc.sync.dma_start(w128[:, j, :], W128_d[j])
        vb = cpool.tile([128, NV, D], BF)
        for j in range(NV):
            nc.sync.dma_start(vb[:, j, :], V256_d[j])
        vb128 = cpool.tile([128, 2, 128], BF)
        for j in range(2):
            nc.sync.dma_start(vb128[:, j, :], V128_d[j])
        ident = cpool.tile([128, 128], F32)
        nc.sync.dma_start(ident[:], id_d[:])
        epsb = cpool.tile([128, 1], F32, name="epsb")
        nc.gpsimd.memset(epsb[:], LN_EPS)

        W = lambda name, kc: wc[:, wi[name] + kc, :]
        V = lambda name: vb[:, vi[name], :]

        # ---- persistent features ----
        xb = fpool.tile([128, NBT, D], F32, tag="xb")
        for j in range(NBT):
            nc.sync.dma_start(xb[:, j, :], xb0_d[j])
        xt = fpool.tile([128, NTT, D], F32, tag="xt")
        for j in range(NTT):
            nc.sync.dma_start(xt[:, j, :], xt0_d[j])

        xbT = fpool.tile([128, 2, BP], BF, tag="xbT")
        xtT = fpool.tile([128, 2, TP], BF, tag="xtT")
        tb_bf = fpool.tile([128, NBT, D], BF, tag="tb_bf")
        ttl_bf = fpool.tile([128, NTT, D], BF, tag="ttl_bf")
        meanT_bf = fpool.tile([128, 2, TP], BF, tag="meanT_bf")

        def transpose_into(dst, src_tile, n_tiles, trp):
            for nt in range(n_tiles):
                for kc in range(2):
                    ps = trp.tile([128, 128], F32, tag="tr")
                    nc.tensor.transpose(ps[:], src_tile[:, nt, ts(kc, 128)], ident[:])
                    if (2 * nt + kc) % 2 == 0:
                        nc.vector.tensor_copy(dst[:, kc, ts(nt, 128)], ps[:])
                    else:
                        nc.scalar.copy(dst[:, kc, ts(nt, 128)], ps[:])

        def ln_epilogue(sb, rowsum, g_ap, b_ap, out_ap):
            mean = tpool.tile([128, 1], F32, tag="mean")
            nc.scalar.mul(mean[:], rowsum[:], 1.0 / D)
            ssq = tpool.tile([128, 1], F32, tag="ssq")
            scr = qpool.tile([128, D], F32, tag="sq")
            nc.scalar.activation(scr[:], sb[:], AF.Square, accum_out=ssq[:])
            m2 = tpool.tile([128, 1], F32, tag="m2")
            nc.scalar.square(m2[:], mean[:])
            var = tpool.tile([128, 1], F32, tag="var")
            nc.vector.scalar_tensor_tensor(var[:], ssq[:], 1.0 / D, m2[:],
                                           ALU.mult, ALU.subtract)
            std = tpool.tile([128, 1], F32, tag="std")
            nc.scalar.activation(std[:], var[:], AF.Sqrt, bias=epsb[:])
            inv = tpool.tile([128, 1], F32, tag="inv")
            nc.vector.reciprocal(inv[:], std[:])
            t1 = epool.tile([128, D], F32, tag="lnt")
            nc.vector.scalar_tensor_tensor(t1[:], sb[:], mean[:], g_ap,
                                           ALU.subtract, ALU.mult)
            nc.vector.scalar_tensor_tensor(out_ap, t1[:], inv[:], b_ap,
                                           ALU.mult, ALU.add)

        def l2_recip(v, width=D, scale=None):
            ssq = tpool.tile([128, 1], F32, tag="l2ssq")
            scr = qpool.tile([128, D], F32, tag="sq")
            nc.scalar.activation(scr[:, :width], v[:], AF.Square, accum_out=ssq[:])
            nc.vector.tensor_scalar_max(ssq[:], ssq[:], 1e-24)
            nrm = tpool.tile([128, 1], F32, tag="l2n")
            nc.scalar.activation(nrm[:], ssq[:], AF.Sqrt)
            rec = tpool.tile([128, 1], F32, tag="l2r")
            nc.vector.reciprocal(rec[:], nrm[:])
            if scale is not None:
                nc.scalar.mul(rec[:], rec[:], scale)
            return rec

        # ================= layers =================
        for i in range(L):
            # ---- phase A: transposes + tb_bf + ttl_bf ----
            with tc.tile_pool(name=f"psA{i}", bufs=4, space="PSUM") as mmA, \
                 tc.tile_pool(name=f"psAt{i}", bufs=2, space="PSUM") as trA:
                transpose_into(xbT, xb, NBT, trA)
                transpose_into(xtT, xt, NTT, trA)
                for nt in range(NBT):
                    ps = mmA.tile([128, D], F32, tag="mm")
                    for kc in range(2):
                        nc.tensor.matmul(ps[:], xbT[:, kc, ts(nt, 128)],
                                         W(("WtT_b", i), kc),
                                         start=kc == 0, stop=kc == 1)
                    nc.vector.scalar_tensor_tensor(tb_bf[:, nt, :], ps[:], 1.0,
                                                   V(("bt_b", i)), ALU.mult, ALU.add)
                for tt_ in range(NTT):
                    ps = mmA.tile([128, D], F32, tag="mm")
                    for kc in range(2):
                        nc.tensor.matmul(ps[:], xtT[:, kc, ts(tt_, 128)],
                                         W(("WctT", i), kc),
                                         start=kc == 0, stop=kc == 1)
                    nc.vector.scalar_tensor_tensor(ttl_bf[:, tt_, :], ps[:], 1.0,
                                                   V(("vttl", i)), ALU.mult, ALU.add)

            # ---- phase B: partial_t (d-major) -> allreduce ----
            pscr = ppool.tile([128, 2, TP], F32, tag="pscr")
            with tc.tile_pool(name=f"psB{i}", bufs=2, space="PSUM") as ptp:
                pt = [ptp.tile([128, TP], F32, tag="pt", name=f"pt{i}_{dh}")
                      for dh in range(2)]
                for c in range(NBT):
                    ab = bpool.tile([128, TP], BF, tag="abt")
                    nc.sync.dma_start(ab[:], Abt_d[c])
                    for dh in range(2):
                        for s in range(4):
                            nc.tensor.matmul(pt[dh][:, ts(s, 512)],
                                             tb_bf[:, c, ts(dh, 128)],
                                             ab[:, ts(s, 512)],
                                             start=c == 0, stop=c == NBT - 1)
                for dh in range(2):
                    for s in range(4):
                        if s % 2 == 0:
                            nc.vector.tensor_copy(pscr[:, dh, ts(s, 512)],
                                                  pt[dh][:, ts(s, 512)])
                        else:
                            nc.scalar.copy(pscr[:, dh, ts(s, 512)],
                                           pt[dh][:, ts(s, 512)])
            bounce_in = dpool.tile([128, 2, TP], F32, tag="bin")
            bounce_out = dpool.tile([128, 2, TP], F32, tag="bout", addr_space="Shared")
            nc.sync.dma_start(bounce_in[:], pscr[:])
            nc.gpsimd.collective_compute(
                "AllReduce", ALU.add, replica_groups=[list(range(NC))],
                ins=[bounce_in.opt()], outs=[bounce_out.opt()])

            # ---- phase C: bacteria aggregation + update ----
            with tc.tile_pool(name=f"psC{i}", bufs=6, space="PSUM") as mmC:
                for ntb in range((NBT + 3) // 4):
                    nts = [ntb * 4 + k for k in range(4) if ntb * 4 + k < NBT]
                    wth = len(nts) * 128
                    pss = {nt: mmC.tile([128, D], F32, tag="mm", name=f"cps{i}_{nt}")
                           for nt in nts}
                    for tck in range(NTT):
                        at = spool.tile([128, 512], BF, tag="at")
                        nc.sync.dma_start(at[:, :wth],
                                          At_d[tck][:, ds(ntb * 512, wth)])
                        for k, nt in enumerate(nts):
                            nc.tensor.matmul(pss[nt][:], at[:, ts(k, 128)],
                                             ttl_bf[:, tck, :],
                                             start=tck == 0, stop=False)
                    for nt in nts:
                        for kc in range(2):
                            nc.tensor.matmul(pss[nt][:], xbT[:, kc, ts(nt, 128)],
                                             W(("WcT_b", i), kc),
                                             start=False, stop=kc == 1)
                        pstb = mmC.tile([128, D], F32, tag="mm")
                        for kc in range(2):
                            nc.tensor.matmul(pstb[:], xbT[:, kc, ts(nt, 128)],
                                             W(("WtT_b", i), kc),
                                             start=kc == 0, stop=kc == 1)
                        cbv = epool.tile([128, D], F32, tag="cbv")
                        nc.vector.scalar_tensor_tensor(cbv[:], pss[nt][:], 1.0,
                                                       V(("blc_b", i)),
                                                       ALU.mult, ALU.add)
                        rec = l2_recip(cbv)
                        s1 = epool.tile([128, D], F32, tag="s1")
                        nc.vector.scalar_tensor_tensor(s1[:], cbv[:], rec[:],
                                                       pstb[:], ALU.mult, ALU.add)
                        sb = epool.tile([128, D], F32, tag="sb")
                        rowsum = tpool.tile([128, 1], F32, tag="rs")
                        nc.vector.scalar_tensor_tensor(sb[:], s1[:], 1.0,
                                                       V(("bt_b", i)),
                                                       ALU.mult, ALU.add,
                                                       accum_out=rowsum[:])
                        ln_epilogue(sb, rowsum, V(("lng_b", i)), V(("lnb_b", i)),
                                    xb[:, nt, :])

            # ---- phase D: trait update ----
            pm = ppool.tile([128, 2, TP], F32, tag="pscr")
            for dh in range(2):
                nc.sync.dma_start(pm[:, dh, :], bounce_out[:, dh, :])
                nc.vector.tensor_copy(meanT_bf[:, dh, :], pm[:, dh, :])
            with tc.tile_pool(name=f"psD{i}", bufs=4, space="PSUM") as mmD:
                for tt_ in range(NTT):
                    ps = mmD.tile([128, D], F32, tag="mm")
                    for kc in range(2):
                        nc.tensor.matmul(ps[:], meanT_bf[:, kc, ts(tt_, 128)],
                                         W(("WlT_t", i), kc),
                                         start=kc == 0, stop=False)
                    for kc in range(2):
                        nc.tensor.matmul(ps[:], xtT[:, kc, ts(tt_, 128)],
                                         W(("WcT_t", i), kc),
                                         start=False, stop=kc == 1)
                    pstt = mmD.tile([128, D], F32, tag="mm")
                    for kc in range(2):
                        nc.tensor.matmul(pstt[:], xtT[:, kc, ts(tt_, 128)],
                                         W(("WtT_t", i), kc),
                                         start=kc == 0, stop=kc == 1)
                    ctv = epool.tile([128, D], F32, tag="cbv")
                    nc.vector.scalar_tensor_tensor(ctv[:], ps[:], 1.0,
                                                   V(("blc_t", i)),
                                                   ALU.mult, ALU.add)
                    rec = l2_recip(ctv)
                    s1 = epool.tile([128, D], F32, tag="s1")
                    nc.vector.scalar_tensor_tensor(s1[:], ctv[:], rec[:],
                                                   pstt[:], ALU.mult, ALU.add)
                    sb = epool.tile([128, D], F32, tag="sb")
                    rowsum = tpool.tile([128, 1], F32, tag="rs")
                    nc.vector.scalar_tensor_tensor(sb[:], s1[:], 1.0,
                                                   V(("bt_t", i)),
                                                   ALU.mult, ALU.add,
                                                   accum_out=rowsum[:])
                    ln_epilogue(sb, rowsum, V(("lng_t", i)), V(("lnb_t", i)),
                                xt[:, tt_, :])

        # ================= metapath + projection + sim =================
        # reuse dead per-layer slots via shared tags
        xtm_bf = fpool.tile([128, NTT, D], BF, tag="ttl_bf")
        mpT_bf = fpool.tile([128, 2, BP], BF, tag="tb_bf")
        htT_bf = fpool.tile([128, 2, TP], BF, tag="meanT_bf")
        hbT_bf = fpool.tile([128, 2, BP], BF, tag="xb")
        hbn_T = fpool.tile([128, BP], BF, tag="xt")
        htn_T = ppool.tile([128, TP], BF, tag="pscr")

        with tc.tile_pool(name="psF1", bufs=4, space="PSUM") as mmF, \
             tc.tile_pool(name="psF1t", bufs=2, space="PSUM") as trF:
            transpose_into(xbT, xb, NBT, trF)
            transpose_into(xtT, xt, NTT, trF)
            for tt_ in range(NTT):
                ps = mmF.tile([128, D], F32, tag="mm")
                for kc in range(2):
                    nc.tensor.matmul(ps[:], xtT[:, kc, ts(tt_, 128)],
                                     W("mpWT", kc), start=kc == 0, stop=kc == 1)
                nc.vector.tensor_copy(xtm_bf[:, tt_, :], ps[:])

        with tc.tile_pool(name="psF2", bufs=6, space="PSUM") as mmZ, \
             tc.tile_pool(name="psF2t", bufs=2, space="PSUM") as trZ:
            for ntb in range((NBT + 3) // 4):
                nts = [ntb * 4 + k for k in range(4) if ntb * 4 + k < NBT]
                wth = len(nts) * 128
                pss = {nt: mmZ.tile([128, D], F32, tag="mm", name=f"zps{nt}")
                       for nt in nts}
                for tck in range(NTT):
                    at = spool.tile([128, 512], BF, tag="at")
                    nc.sync.dma_start(at[:, :wth],
                                      Amp_d[tck][:, ds(ntb * 512, wth)])
                    for k, nt in enumerate(nts):
                        nc.tensor.matmul(pss[nt][:], at[:, ts(k, 128)],
                                         xtm_bf[:, tck, :],
                                         start=tck == 0, stop=False)
                for nt in nts:
                    for kc in range(2):
                        nc.tensor.matmul(pss[nt][:], xbT[:, kc, ts(nt, 128)],
                                         W("mpWT_x", kc), start=False,
                                         stop=kc == 1)
                    zv = epool.tile([128, D], F32, tag="cbv")
                    rowsum = tpool.tile([128, 1], F32, tag="rs")
                    nc.vector.scalar_tensor_tensor(zv[:], pss[nt][:], 1.0,
                                                   V("mpb"), ALU.mult, ALU.add,
                                                   accum_out=rowsum[:])
                    mpo = epool.tile([128, D], F32, tag="sb")
                    ln_epilogue(zv, rowsum, V("mplng"), V("mplnb"), mpo[:])
                    for kc in range(2):
                        pst = trZ.tile([128, 128], F32, tag="tr")
                        nc.tensor.transpose(pst[:], mpo[:, ts(kc, 128)], ident[:])
                        nc.vector.tensor_copy(mpT_bf[:, kc, ts(nt, 128)], pst[:])

        with tc.tile_pool(name="psF3", bufs=6, space="PSUM") as mmP, \
             tc.tile_pool(name="psF3t", bufs=2, space="PSUM") as trP:
            for nt in range(NBT):
                ps = mmP.tile([128, D], F32, tag="mm")
                for kc in range(2):
                    nc.tensor.matmul(ps[:], xbT[:, kc, ts(nt, 128)],
                                     W("Wp1baT", kc), start=kc == 0, stop=False)
                for kc in range(2):
                    nc.tensor.matmul(ps[:], mpT_bf[:, kc, ts(nt, 128)],
                                     W("Wp1bbT", kc), start=False, stop=kc == 1)
                hv = epool.tile([128, D], F32, tag="cbv")
                rowsum = tpool.tile([128, 1], F32, tag="rs")
                nc.vector.scalar_tensor_tensor(hv[:], ps[:], 1.0, V("bp1b"),
                                               ALU.mult, ALU.add,
                                               accum_out=rowsum[:])
                lno = epool.tile([128, D], F32, tag="sb")
                ln_epilogue(hv, rowsum, V("plngb"), V("plnbb"), lno[:])
                hbr = epool.tile([128, D], F32, tag="s1")
                nc.scalar.activation(hbr[:], lno[:], AF.Relu)
                for kc in range(2):
                    pst = trP.tile([128, 128], F32, tag="tr")
                    nc.tensor.transpose(pst[:], hbr[:, ts(kc, 128)], ident[:])
                    nc.vector.tensor_copy(hbT_bf[:, kc, ts(nt, 128)], pst[:])
            for tt_ in range(NTT):
                ps = mmP.tile([128, D], F32, tag="mm")
                for kc in range(2):
                    nc.tensor.matmul(ps[:], xtT[:, kc, ts(tt_, 128)],
                                     W("Wp1tT", kc), start=kc == 0, stop=kc == 1)
                hv = epool.tile([128, D], F32, tag="cbv")
                rowsum = tpool.tile([128, 1], F32, tag="rs")
                nc.vector.scalar_tensor_tensor(hv[:], ps[:], 1.0, V("bp1t"),
                                               ALU.mult, ALU.add,
                                               accum_out=rowsum[:])
                lno = epool.tile([128, D], F32, tag="sb")
                ln_epilogue(hv, rowsum, V("plngt"), V("plnbt"), lno[:])
                htr = epool.tile([128, D], F32, tag="s1")
                nc.scalar.activation(htr[:], lno[:], AF.Relu)
                for kc in range(2):
                    pst = trP.tile([128, 128], F32, tag="tr")
                    nc.tensor.transpose(pst[:], htr[:, ts(kc, 128)], ident[:])
                    nc.vector.tensor_copy(htT_bf[:, kc, ts(tt_, 128)], pst[:])
            for nt in range(NBT):
                ps = mmP.tile([128, 128], F32, tag="mm")
                for kc in range(2):
                    nc.tensor.matmul(ps[:], hbT_bf[:, kc, ts(nt, 128)],
                                     w128[:, w128i["Wp2bT"] + kc, :],
                                     start=kc == 0, stop=kc == 1)
                hv = epool.tile([128, 128], F32, tag="h2")
                nc.vector.scalar_tensor_tensor(hv[:], ps[:, :128], 1.0,
                                               vb128[:, v128i["bp2b"], :],
                                               ALU.mult, ALU.add)
                rec = l2_recip(hv, width=128)
                hn = epool.tile([128, 128], F32, tag="h2n")
                nc.scalar.activation(hn[:], hv[:], AF.Copy, scale=rec[:])
                pst = trP.tile([128, 128], F32, tag="tr")
                nc.tensor.transpose(pst[:], hn[:], ident[:])
                nc.vector.tensor_copy(hbn_T[:, ts(nt, 128)], pst[:])
            for tt_ in range(NTT):
                ps = mmP.tile([128, 128], F32, tag="mm")
                for kc in range(2):
                    nc.tensor.matmul(ps[:], htT_bf[:, kc, ts(tt_, 128)],
                                     w128[:, w128i["Wp2tT"] + kc, :],
                                     start=kc == 0, stop=kc == 1)
                hv = epool.tile([128, 128], F32, tag="h2")
                nc.vector.scalar_tensor_tensor(hv[:], ps[:, :128], 1.0,
                                               vb128[:, v128i["bp2t"], :],
                                               ALU.mult, ALU.add)
                rec = l2_recip(hv, width=128, scale=temp)
                hn = epool.tile([128, 128], F32, tag="h2n")
                nc.scalar.activation(hn[:], hv[:], AF.Copy, scale=rec[:])
                pst = trP.tile([128, 128], F32, tag="tr")
                nc.tensor.transpose(pst[:], hn[:], ident[:])
                nc.vector.tensor_copy(htn_T[:, ts(tt_, 128)], pst[:])

        with tc.tile_pool(name="psS", bufs=4, space="PSUM") as mmS:
            for nt in range(NBT):
                for s in range(4):
                    ps = mmS.tile([128, 512], F32, tag="sim")
                    nc.tensor.matmul(ps[:], hbn_T[:, ts(nt, 128)],
                                     htn_T[:, ts(s, 512)], start=True, stop=True)
                    ob = epool.tile([128, 512], F32, tag="simout")
                    if s % 2 == 0:
                        nc.vector.tensor_copy(ob[:], ps[:])
                    else:
                        nc.scalar.copy(ob[:], ps[:])
                    nc.sync.dma_start(sim_d[nt][:, ts(s, 512)], ob[:])

    nc.compile()
    return nc


# ---------------------------------------------------------------------------
# Entry point
# ---------------------------------------------------------------------------

def kernel(**inputs):
    in_maps, meta = _prep(inputs)
    nc = build_program(meta)
    res = run_bass_kernel_spmd(nc, in_maps, core_ids=list(range(NC)))
    sim = np.empty((N_B, N_T), np.float32)
    for c in range(NC):
        shard = np.asarray(res.results[c]["simO"], np.float32).reshape(BP, TP)
        sim[c * B_SH:(c + 1) * B_SH] = shard[:B_SH, :N_T]
    if meta["simb"] != 0.0:
        sim += np.float32(meta["simb"])
    return sim



# revision 20
# speedup vs baseline: 1.1962x; 1.1962x over previous
"""Trainium2 Bass kernel for nn_HCMGNNBasedMetaPathModel.

Strategy: bacteria nodes sharded over 8 cores (3750 -> padded 3840 rows each);
trait side + weights replicated. Edge-list segment ops are reformulated as
dense (deg-normalized) adjacency matmuls built on the host in bf16.
Per layer the trait-side partial aggregates are combined with a bf16
ReduceScatter (which also carries a 1/8-scaled copy of the core's own xtT
block, so the trait update needs no rank-dependent addressing), the trait
update runs sharded 2 tiles/core, and a bf16 AllGather rebuilds xt.
On-device layout is node-major [128-row tiles, 256 free]; feature
contractions take a d-major (PE-transposed) bf16 copy as the stationary
operand. Linear-algebra folds remove all other transposes:
  tb@Wr.T = xb@(Wr@Wt).T + (Wr@bt)      (lin_r branch)
  ttl     = xt@(Wl_b@Wt_t).T + Wl_b@bt_t (pre-multiplied neighbor features)
  mean scaling folded into adjacency rows (host), counts exact there.
Residual tb comes from the stored bf16 tb tile (no recompute), LayerNorm
stats use the bn_stats/bn_aggr DVE instructions, and the final phase runs
trait-side first then a fused per-node-tile-group pipeline
(metapath agg -> LN -> proj1 -> relu -> proj2 -> l2 -> sim matmul -> f16 out)
so the similarity output DMA overlaps projection compute.
"""
import contextlib
import sys

for _p in ("/opt/trn_rl_repo",):
    if _p not in sys.path:
        sys.path.insert(0, _p)

import numpy as np
import ml_dtypes

import concourse.bass as bass
import concourse.tile as tile
from concourse import bacc, mybir
from concourse.bass_utils import run_bass_kernel_spmd

BF16 = ml_dtypes.bfloat16
F32 = mybir.dt.float32
F16 = mybir.dt.float16
BF = mybir.dt.bfloat16
AF = mybir.ActivationFunctionType
ALU = mybir.AluOpType
ts, ds = bass.ts, bass.ds

N_B, N_T, D, L, M = 30000, 2000, 256, 3, 2
NC = 8
B_SH = 3750          # real bacteria rows per core
BP = 3840            # padded bacteria rows per core
NBT = BP // 128      # 30 node tiles
TP = 2048            # padded trait rows
NTT = TP // 128      # 16 trait tiles
TB = TP // NC        # 256 traits per core (2 tiles) for the sharded update
LN_EPS = 1e-5

# ---------------------------------------------------------------------------
# Host-side preprocessing
# ---------------------------------------------------------------------------

def _dense_adj(src, dst, n_dst, n_src):
    """A[d, s] = #edges (s->d), rows scaled by 1/max(deg,1)."""
    idx = dst.astype(np.int64) * n_src + src.astype(np.int64)
    A = np.bincount(idx, minlength=n_dst * n_src).astype(np.float32)
    A = A.reshape(n_dst, n_src)
    deg = np.bincount(dst.astype(np.int64), minlength=n_dst).astype(np.float32)
    A *= (1.0 / np.maximum(deg, 1.0))[:, None]
    return A


def _prep(inp):
    f32 = np.float32
    emb_b = np.asarray(inp["emb_b"], f32)
    emb_t = np.asarray(inp["emb_t"], f32)

    A_tb = _dense_adj(np.asarray(inp["src_tb"]), np.asarray(inp["dst_tb"]), N_B, N_T)
    A_bt = _dense_adj(np.asarray(inp["src_bt"]), np.asarray(inp["dst_bt"]), N_T, N_B)
    mpw = np.asarray(inp["mp_w"], np.float64)
    e = np.exp(mpw - mpw.max())
    w = e / e.sum()
    sw = float(w.sum())
    mp_adj = np.asarray(inp["mp_adj"], f32)
    A_mp = (w[0] * mp_adj[0].astype(np.float64) +
            w[1] * mp_adj[1].astype(np.float64)).astype(f32)

    xb0 = np.zeros((NC, NBT, 128, D), BF16)
    xb0.reshape(NC, BP, D)[:, :B_SH] = emb_b.reshape(NC, B_SH, D).astype(BF16)
    xt0 = np.zeros((NTT, 128, D), BF16)
    xt0.reshape(TP, D)[:N_T] = emb_t.astype(BF16)

    def shard_T(A):  # [N_B, N_T] -> per-core [NTT, 128, BP] trait-major bf16
        out = np.zeros((NC, NTT, 128, BP), BF16)
        for c in range(NC):
            blk = np.zeros((TP, BP), f32)
            blk[:N_T, :B_SH] = A[c * B_SH:(c + 1) * B_SH].T
            out[c] = blk.reshape(NTT, 128, BP).astype(BF16)
        return out

    At = shard_T(A_tb)
    Amp = shard_T(A_mp)
    Abt = np.zeros((NC, NBT, 128, TP), BF16)
    for c in range(NC):
        blk = np.zeros((BP, TP), f32)
        blk[:B_SH, :N_T] = A_bt[:, c * B_SH:(c + 1) * B_SH].T
        Abt[c] = blk.reshape(NBT, 128, TP).astype(BF16)

    Wt_b, bt_b = np.asarray(inp["Wt_b"], f32), np.asarray(inp["bt_b"], f32)
    Wt_t, bt_t = np.asarray(inp["Wt_t"], f32), np.asarray(inp["bt_t"], f32)
    Wl_b, bl_b = np.asarray(inp["Wl_b"], f32), np.asarray(inp["bl_b"], f32)
    Wr_b = np.asarray(inp["Wr_b"], f32)
    Wl_t, bl_t = np.asarray(inp["Wl_t"], f32), np.asarray(inp["bl_t"], f32)
    Wr_t = np.asarray(inp["Wr_t"], f32)

    wlist, wi = [], {}

    def addW(name, WT):
        wi[name] = len(wlist)
        for kc in range(WT.shape[0] // 128):
            wlist.append(np.ascontiguousarray(WT[kc * 128:(kc + 1) * 128]).astype(BF16))

    for i in range(L):
        addW(("WtT_b", i), Wt_b[i].T)
        addW(("WcT_b", i), (Wr_b[i] @ Wt_b[i]).T)
        addW(("WctT", i), (Wl_b[i] @ Wt_t[i]).T)
        addW(("WtT_t", i), Wt_t[i].T)
        addW(("WcT_t", i), (Wr_t[i] @ Wt_t[i]).T)
        addW(("WlT_t", i), Wl_t[i].T)
    mpW = np.asarray(inp["mpW"], f32)
    Wp1b = np.asarray(inp["Wp1b"], f32)
    Wp1t = np.asarray(inp["Wp1t"], f32)
    # interleaved [mpWT | Wp1tT] per k-chunk so one n=512 matmul computes both
    mpWT, Wp1tT = mpW.T, Wp1t.T
    wi["mpW_p1t"] = len(wlist)
    for kc in range(2):
        wlist.append(np.ascontiguousarray(mpWT[kc * 128:(kc + 1) * 128]).astype(BF16))
        wlist.append(np.ascontiguousarray(Wp1tT[kc * 128:(kc + 1) * 128]).astype(BF16))
    addW("mpWT_x", sw * mpW.T)
    addW("Wp1baT", Wp1b[:, :D].T)
    addW("Wp1bbT", Wp1b[:, D:].T)
    Wc256 = np.stack(wlist)

    w128list, w128i = [], {}

    def addW128(name, WT):
        w128i[name] = len(w128list)
        for kc in range(WT.shape[0] // 128):
            w128list.append(np.ascontiguousarray(WT[kc * 128:(kc + 1) * 128]).astype(BF16))

    addW128("Wp2bT", np.asarray(inp["Wp2b"], f32).T)
    addW128("Wp2tT", np.asarray(inp["Wp2t"], f32).T)
    W128 = np.stack(w128list)

    vlist, vi = [], {}

    def addV(name, v):
        vi[name] = len(vlist)
        vlist.append(np.ascontiguousarray(
            np.broadcast_to(v.astype(f32), (128, v.shape[0]))).astype(BF16))

    for i in range(L):
        addV(("bt_b", i), bt_b[i])
        addV(("blc_b", i), bl_b[i] + Wr_b[i] @ bt_b[i])
        addV(("vttl", i), Wl_b[i] @ bt_t[i])
        addV(("lng_b", i), np.asarray(inp["lng_b"], f32)[i])
        addV(("lnb_b", i), np.asarray(inp["lnb_b"], f32)[i])
        addV(("bt_t", i), bt_t[i])
        addV(("blc_t", i), bl_t[i] + Wr_t[i] @ bt_t[i])
        addV(("lng_t", i), np.asarray(inp["lng_t"], f32)[i])
        addV(("lnb_t", i), np.asarray(inp["lnb_t"], f32)[i])
    for nm in ("mpb", "mplng", "mplnb", "bp1b", "plngb", "plnbb",
               "bp1t", "plngt", "plnbt"):
        addV(nm, np.asarray(inp[nm], f32))
    V256 = np.stack(vlist)

    v128list, v128i = [], {}
    for nm in ("bp2b", "bp2t"):
        v128i[nm] = len(v128list)
        v128list.append(np.ascontiguousarray(
            np.broadcast_to(np.asarray(inp[nm], f32), (128, 128))).astype(BF16))
    V128 = np.stack(v128list)

    ident = np.eye(128, dtype=f32).astype(BF16)
    temp = float(np.asarray(inp["temperature"]).reshape(-1)[0])
    simb = float(np.asarray(inp["sim_bias"]).reshape(-1)[0])

    shared = dict(xt0=xt0, Wc256=Wc256, W128=W128, V256=V256, V128=V128,
                  ident=ident)
    in_maps = []
    for c in range(NC):
        m = dict(shared)
        m["xb0"] = np.ascontiguousarray(xb0[c])
        m["At"] = np.ascontiguousarray(At[c])
        m["Abt"] = np.ascontiguousarray(Abt[c])
        m["Amp"] = np.ascontiguousarray(Amp[c])
        in_maps.append(m)
    meta = dict(wi=wi, w128i=w128i, vi=vi, v128i=v128i,
                wcount=len(wlist), vcount=len(vlist), temp=temp, simb=simb)
    return in_maps, meta


# ---------------------------------------------------------------------------
# Device program
# ---------------------------------------------------------------------------

def build_program(meta):
    nc = bacc.Bacc("TRN2", target_bir_lowering=False, debug=False,
                   num_devices=NC)
    wi, w128i, vi, v128i = meta["wi"], meta["w128i"], meta["vi"], meta["v128i"]
    NW, NV = meta["wcount"], meta["vcount"]
    temp = meta["temp"]

    xb0_d = nc.dram_tensor("xb0", [NBT, 128, D], BF, kind="ExternalInput")
    xt0_d = nc.dram_tensor("xt0", [NTT, 128, D], BF, kind="ExternalInput")
    At_d = nc.dram_tensor("At", [NTT, 128, BP], BF, kind="ExternalInput")
    Abt_d = nc.dram_tensor("Abt", [NBT, 128, TP], BF, kind="ExternalInput")
    Amp_d = nc.dram_tensor("Amp", [NTT, 128, BP], BF, kind="ExternalInput")
    Wc_d = nc.dram_tensor("Wc256", [NW, 128, D], BF, kind="ExternalInput")
    W128_d = nc.dram_tensor("W128", [4, 128, 128], BF, kind="ExternalInput")
    V256_d = nc.dram_tensor("V256", [NV, 128, D], BF, kind="ExternalInput")
    V128_d = nc.dram_tensor("V128", [2, 128, 128], BF, kind="ExternalInput")
    id_d = nc.dram_tensor("ident", [128, 128], BF, kind="ExternalInput")
    sim_d = nc.dram_tensor("simO", [NBT, 128, TP], F16, kind="ExternalOutput")

    with tile.TileContext(nc) as tc, contextlib.ExitStack() as ctx:
        cpool = ctx.enter_context(tc.tile_pool(name="const", bufs=1))
        fpool = ctx.enter_context(tc.tile_pool(name="feat", bufs=1))
        spool = ctx.enter_context(tc.tile_pool(name="stream", bufs=4))
        bpool = ctx.enter_context(tc.tile_pool(name="abt_stream", bufs=3))
        epool = ctx.enter_context(tc.tile_pool(name="epi", bufs=4))
        qpool = ctx.enter_context(tc.tile_pool(name="sq", bufs=3))
        tpool = ctx.enter_context(tc.tile_pool(name="tiny", bufs=10))
        opool = ctx.enter_context(tc.tile_pool(name="simout", bufs=4))
        dpool = ctx.enter_context(tc.tile_pool(name="dram", bufs=2, space="DRAM"))

        # ---- persistent features (loaded first so layer 0 starts ASAP) ----
        ident = cpool.tile([128, 128], BF)
        nc.sync.dma_start(ident[:], id_d[:])
        xb = fpool.tile([128, NBT, D], BF, tag="xb")
        for j in range(NBT):
            nc.sync.dma_start(xb[:, j, :], xb0_d[j])
        wc = cpool.tile([128, NW, D], BF)
        for j in range(NW):
            nc.sync.dma_start(wc[:, j, :], Wc_d[j])
        vb = cpool.tile([128, NV, D], BF)
        for j in range(NV):
            nc.sync.dma_start(vb[:, j, :], V256_d[j])
        xt = fpool.tile([128, NTT, D], BF, tag="xt")
        for j in range(NTT):
            nc.sync.dma_start(xt[:, j, :], xt0_d[j])
        w128 = cpool.tile([128, 4, 128], BF)
        for j in range(4):
            nc.sync.dma_start(w128[:, j, :], W128_d[j])
        vb128 = cpool.tile([128, 2, 128], BF)
        for j in range(2):
            nc.sync.dma_start(vb128[:, j, :], V128_d[j])
        epsb = cpool.tile([128, 1], F32, name="epsb")
        nc.gpsimd.memset(epsb[:], LN_EPS)
        eps24 = cpool.tile([128, 1], F32, name="eps24")
        nc.gpsimd.memset(eps24[:], 1e-24)

        W = lambda name, kc: wc[:, wi[name] + kc, :]
        V = lambda name: vb[:, vi[name], :]

        xbT = fpool.tile([128, 2, BP], BF, tag="xbT")
        xtT = fpool.tile([128, 2, TP], BF, tag="xtT")
        tb_bf = fpool.tile([128, NBT, D], BF, tag="tb_bf")
        ttl_bf = fpool.tile([128, NTT, D], BF, tag="ttl_bf")
        ptb_bf = fpool.tile([128, 2, TP], BF, tag="ptb_bf")
        pm_sb = fpool.tile([128, 4, TB], BF, tag="pm_sb")
        xtO_T = fpool.tile([128, 2, TB], BF, tag="xtO_T")

        def transpose_into(dst, src_tile, n_tiles, trp):
            for nt in range(n_tiles):
                for kc in range(2):
                    ps = trp.tile([128, 128], BF, tag="tr")
                    nc.tensor.transpose(ps[:], src_tile[:, nt, ts(kc, 128)], ident[:])
                    if (2 * nt + kc) % 2 == 0:
                        nc.vector.tensor_copy(dst[:, kc, ts(nt, 128)], ps[:])
                    else:
                        nc.scalar.copy(dst[:, kc, ts(nt, 128)], ps[:])

        def ln_epilogue(sb, g_ap, b_ap, out_ap):
            st6 = tpool.tile([128, 6], F32, tag="st6")
            nc.vector.bn_stats(st6[:], sb[:])
            mv = tpool.tile([128, 2], F32, tag="mv")
            nc.vector.bn_aggr(mv[:], st6[:])
            std = tpool.tile([128, 1], F32, tag="std")
            nc.scalar.activation(std[:], mv[:, 1:2], AF.Sqrt, bias=epsb[:])
            inv = tpool.tile([128, 1], F32, tag="inv")
            nc.vector.reciprocal(inv[:], std[:])
            t1 = epool.tile([128, D], F32, tag="lnt")
            nc.vector.scalar_tensor_tensor(t1[:], sb[:], mv[:, 0:1], g_ap,
                                           ALU.subtract, ALU.mult)
            nc.vector.scalar_tensor_tensor(out_ap, t1[:], inv[:], b_ap,
                                           ALU.mult, ALU.add)

        def l2_recip(v, width=D, scale=None):
            ssq = tpool.tile([128, 1], F32, tag="l2ssq")
            scr = qpool.tile([128, D], F32, tag="sq")
            nc.scalar.activation(scr[:, :width], v[:], AF.Square, accum_out=ssq[:])
            nrm = tpool.tile([128, 1], F32, tag="l2n")
            nc.scalar.activation(nrm[:], ssq[:], AF.Sqrt, bias=eps24[:])
            rec = tpool.tile([128, 1], F32, tag="l2r")
            nc.vector.reciprocal(rec[:], nrm[:])
            if scale is not None:
                nc.scalar.mul(rec[:], rec[:], scale)
            return rec

        # ================= layers =================
        for i in range(L):
            # ---- phase A: xb transposes + tb, then xt transposes + ttl ----
            with tc.tile_pool(name=f"psA{i}", bufs=4, space="PSUM") as mmA, \
                 tc.tile_pool(name=f"psAt{i}", bufs=2, space="PSUM") as trA:
                transpose_into(xbT, xb, NBT, trA)
                for nt in range(NBT):
                    ps = mmA.tile([128, D], F32, tag="mm")
                    for kc in range(2):
                        nc.tensor.matmul(ps[:], xbT[:, kc, ts(nt, 128)],
                                         W(("WtT_b", i), kc),
                                         start=kc == 0, stop=kc == 1)
                    nc.vector.scalar_tensor_tensor(tb_bf[:, nt, :], ps[:], 1.0,
                                                   V(("bt_b", i)), ALU.mult, ALU.add)
                transpose_into(xtT, xt, NTT, trA)
                for tt_ in range(NTT):
                    ps = mmA.tile([128, D], F32, tag="mm")
                    for kc in range(2):
                        nc.tensor.matmul(ps[:], xtT[:, kc, ts(tt_, 128)],
                                         W(("WctT", i), kc),
                                         start=kc == 0, stop=kc == 1)
                    nc.vector.scalar_tensor_tensor(ttl_bf[:, tt_, :], ps[:], 1.0,
                                                   V(("vttl", i)), ALU.mult, ALU.add)

            # RS payload: per target core k -> [partial_dh0, partial_dh1,
            # xtT_k_kc0, xtT_k_kc1] each [128, TB].  The xtT blocks are summed
            # 8x by the ReduceScatter; the consumer rescales by 1/8 (exact).
            rs_in = dpool.tile([NC, 4, 128, TB], BF, tag="rs_in")
            rs_out = dpool.tile([4, 128, TB], BF, tag="rs_out")
            ag_in = dpool.tile([2, 128, D], BF, tag="ag_in")
            ag_out = dpool.tile([NTT, 128, D], BF, tag="ag_out",
                                addr_space="Shared")
            for k in range(NC):
                for kc in range(2):
                    nc.sync.dma_start(rs_in[k, 2 + kc], xtT[:, kc, ts(k, TB)])

            # ---- phase B: partial_t (d-major) -> RS ----
            with tc.tile_pool(name=f"psB{i}", bufs=2, space="PSUM") as ptp:
                pt = [ptp.tile([128, TP], F32, tag="pt", name=f"pt{i}_{dh}")
                      for dh in range(2)]
                for c in range(NBT):
                    ab = bpool.tile([128, TP], BF, tag="abt")
                    nc.sync.dma_start(ab[:], Abt_d[c])
                    for dh in range(2):
                        for s in range(4):
                            nc.tensor.matmul(pt[dh][:, ts(s, 512)],
                                             tb_bf[:, c, ts(dh, 128)],
                                             ab[:, ts(s, 512)],
                                             start=c == 0, stop=c == NBT - 1)
                for dh in range(2):
                    for s in range(4):
                        if s % 2 == 0:
                            nc.vector.tensor_copy(ptb_bf[:, dh, ts(s, 512)],
                                                  pt[dh][:, ts(s, 512)])
                        else:
                            nc.scalar.copy(ptb_bf[:, dh, ts(s, 512)],
                                           pt[dh][:, ts(s, 512)])
            for k in range(NC):
                for dh in range(2):
                    nc.sync.dma_start(rs_in[k, dh], ptb_bf[:, dh, ts(k, TB)])
            nc.gpsimd.collective_compute(
                "ReduceScatter", ALU.add, replica_groups=[list(range(NC))],
                ins=[rs_in.opt()], outs=[rs_out.opt()])

            # ---- phase D (emitted mid-C): sharded trait update + AG ----
            def emit_D():
                for h in range(4):
                    nc.sync.dma_start(pm_sb[:, h, :], rs_out[h])
                nc.vector.tensor_scalar_mul(xtO_T[:], pm_sb[:, 2:4, :], 0.125)
                with tc.tile_pool(name=f"psD{i}", bufs=2, space="PSUM") as mmD:
                    for j in range(2):
                        ps = mmD.tile([128, D], F32, tag="mm")
                        for kc in range(2):
                            nc.tensor.matmul(ps[:], pm_sb[:, kc, ts(j, 128)],
                                             W(("WlT_t", i), kc),
                                             start=kc == 0, stop=False)
                        for kc in range(2):
                            nc.tensor.matmul(ps[:], xtO_T[:, kc, ts(j, 128)],
                                             W(("WcT_t", i), kc),
                                             start=False, stop=kc == 1)
                        pstt = mmD.tile([128, D], F32, tag="mm")
                        for kc in range(2):
                            nc.tensor.matmul(pstt[:], xtO_T[:, kc, ts(j, 128)],
                                             W(("WtT_t", i), kc),
                                             start=kc == 0, stop=kc == 1)
                        ctv = epool.tile([128, D], F32, tag="cbv")
                        nc.vector.scalar_tensor_tensor(ctv[:], ps[:], 1.0,
                                                       V(("blc_t", i)),
                                                       ALU.mult, ALU.add)
                        rec = l2_recip(ctv)
                        s1 = epool.tile([128, D], F32, tag="s1")
                        nc.vector.scalar_tensor_tensor(s1[:], ctv[:], rec[:],
                                                       pstt[:], ALU.mult, ALU.add)
                        sb = epool.tile([128, D], F32, tag="sb")
                        nc.vector.scalar_tensor_tensor(sb[:], s1[:], 1.0,
                                                       V(("bt_t", i)),
                                                       ALU.mult, ALU.add)
                        xtn = epool.tile([128, D], BF, tag="xtn")
                        ln_epilogue(sb, V(("lng_t", i)), V(("lnb_t", i)), xtn[:])
                        nc.sync.dma_start(ag_in[j], xtn[:])
                nc.gpsimd.collective_compute(
                    "AllGather", ALU.bypass, replica_groups=[list(range(NC))],
                    ins=[ag_in.opt()], outs=[ag_out.opt()])
                for j in range(NTT):
                    nc.sync.dma_start(xt[:, j, :], ag_out[j])

            # ---- phase C: bacteria aggregation + update ----
            with tc.tile_pool(name=f"psC{i}", bufs=6, space="PSUM") as mmC:
                for ntb in range((NBT + 3) // 4):
                    nts = [ntb * 4 + k for k in range(4) if ntb * 4 + k < NBT]
                    wth = len(nts) * 128
                    pss = {nt: mmC.tile([128, D], F32, tag="mm", name=f"cps{i}_{nt}")
                           for nt in nts}
                    for tck in range(NTT):
                        at = spool.tile([128, 512], BF, tag="at")
                        nc.sync.dma_start(at[:, :wth],
                                          At_d[tck][:, ds(ntb * 512, wth)])
                        for k, nt in enumerate(nts):
                            nc.tensor.matmul(pss[nt][:], at[:, ts(k, 128)],
                                             ttl_bf[:, tck, :],
                                             start=tck == 0, stop=False)
                    for nt in nts:
                        for kc in range(2):
                            nc.tensor.matmul(pss[nt][:], xbT[:, kc, ts(nt, 128)],
                                             W(("WcT_b", i), kc),
                                             start=False, stop=kc == 1)
                        cbv = epool.tile([128, D], F32, tag="cbv")
                        nc.vector.scalar_tensor_tensor(cbv[:], pss[nt][:], 1.0,
                                                       V(("blc_b", i)),
                                                       ALU.mult, ALU.add)
                        rec = l2_recip(cbv)
                        sb = epool.tile([128, D], F32, tag="sb")
                        nc.vector.scalar_tensor_tensor(sb[:], cbv[:], rec[:],
                                                       tb_bf[:, nt, :],
                                                       ALU.mult, ALU.add)
                        ln_epilogue(sb, V(("lng_b", i)), V(("lnb_b", i)),
                                    xb[:, nt, :])
                    if ntb == 3:
                        emit_D()

        # ================= metapath + projection + sim =================
        xtm_bf = fpool.tile([128, NTT, D], BF, tag="ttl_bf")
        htn_T = fpool.tile([128, TP], BF, tag="ptb_bf")

        with tc.tile_pool(name="psF1", bufs=3, space="PSUM") as mmF, \
             tc.tile_pool(name="psF1s", bufs=2, space="PSUM") as mmG, \
             tc.tile_pool(name="psF1t", bufs=2, space="PSUM") as trF:
            transpose_into(xbT, xb, NBT, trF)
            transpose_into(xtT, xt, NTT, trF)
            # merged [xtm | ht1] in one n=512 matmul per k-chunk
            for tt_ in range(NTT):
                ps = mmF.tile([128, 2 * D], F32, tag="mmw")
                for kc in range(2):
                    nc.tensor.matmul(ps[:], xtT[:, kc, ts(tt_, 128)],
                                     wc[:, ds(wi["mpW_p1t"] + 2 * kc, 2), :],
                                     start=kc == 0, stop=kc == 1)
                nc.vector.tensor_copy(xtm_bf[:, tt_, :], ps[:, 0:D])
                hv = epool.tile([128, D], F32, tag="cbv")
                nc.vector.scalar_tensor_tensor(hv[:], ps[:, D:2 * D], 1.0,
                                               V("bp1t"), ALU.mult, ALU.add)
                lno = epool.tile([128, D], F32, tag="sb")
                ln_epilogue(hv, V("plngt"), V("plnbt"), lno[:])
                htr = epool.tile([128, D], BF, tag="relu_bf")
                nc.scalar.activation(htr[:], lno[:], AF.Relu)
                htT = epool.tile([128, 2, 128], BF, tag="htT")
                for kc in range(2):
                    pst = trF.tile([128, 128], BF, tag="tr")
                    nc.tensor.transpose(pst[:], htr[:, ts(kc, 128)], ident[:])
                    nc.vector.tensor_copy(htT[:, kc, :], pst[:])
                ps2 = mmG.tile([128, 128], F32, tag="mms")
                for kc in range(2):
                    nc.tensor.matmul(ps2[:], htT[:, kc, :],
                                     w128[:, w128i["Wp2tT"] + kc, :],
                                     start=kc == 0, stop=kc == 1)
                hv2 = epool.tile([128, 128], F32, tag="h2")
                nc.vector.scalar_tensor_tensor(hv2[:], ps2[:], 1.0,
                                               vb128[:, v128i["bp2t"], :],
                                               ALU.mult, ALU.add)
                rec = l2_recip(hv2, width=128, scale=temp)
                hn = epool.tile([128, 128], BF, tag="h2n")
                nc.scalar.activation(hn[:], hv2[:], AF.Copy, scale=rec[:])
                pst = trF.tile([128, 128], BF, tag="tr")
                nc.tensor.transpose(pst[:], hn[:], ident[:])
                nc.vector.tensor_copy(htn_T[:, ts(tt_, 128)], pst[:])

        # fused bacteria pipeline: metapath agg -> LN -> proj1 -> relu ->
        # proj2 -> l2 -> sim -> f16 out
        SIMW = [512, 512, 512, N_T - 3 * 512]
        with tc.tile_pool(name="psF2", bufs=4, space="PSUM") as mmZ, \
             tc.tile_pool(name="psF2p", bufs=2, space="PSUM") as mmP, \
             tc.tile_pool(name="psF2t", bufs=2, space="PSUM") as trZ:
            for ntb in range((NBT + 3) // 4):
                nts = [ntb * 4 + k for k in range(4) if ntb * 4 + k < NBT]
                wth = len(nts) * 128
                pss = {nt: mmZ.tile([128, D], F32, tag="mm", name=f"zps{nt}")
                       for nt in nts}
                for tck in range(NTT):
                    at = spool.tile([128, 512], BF, tag="at")
                    nc.sync.dma_start(at[:, :wth],
                                      Amp_d[tck][:, ds(ntb * 512, wth)])
                    for k, nt in enumerate(nts):
                        nc.tensor.matmul(pss[nt][:], at[:, ts(k, 128)],
                                         xtm_bf[:, tck, :],
                                         start=tck == 0, stop=False)
                for nt in nts:
                    for kc in range(2):
                        nc.tensor.matmul(pss[nt][:], xbT[:, kc, ts(nt, 128)],
                                         W("mpWT_x", kc), start=False,
                                         stop=kc == 1)
                    zv = epool.tile([128, D], F32, tag="cbv")
                    nc.vector.scalar_tensor_tensor(zv[:], pss[nt][:], 1.0,
                                                   V("mpb"), ALU.mult, ALU.add)
                    mpo = epool.tile([128, D], BF, tag="mpo_bf")
                    ln_epilogue(zv, V("mplng"), V("mplnb"), mpo[:])
                    mpT = epool.tile([128, 2, 128], BF, tag="mpT")
                    for kc in range(2):
                        pst = trZ.tile([128, 128], BF, tag="tr")
                        nc.tensor.transpose(pst[:], mpo[:, ts(kc, 128)], ident[:])
                        nc.vector.tensor_copy(mpT[:, kc, :], pst[:])
                    ps1 = mmP.tile([128, 2 * D], F32, tag="mmp")
                    for kc in range(2):
                        nc.tensor.matmul(ps1[:, :D], xbT[:, kc, ts(nt, 128)],
                                         W("Wp1baT", kc), start=kc == 0,
                                         stop=False)
                    for kc in range(2):
                        nc.tensor.matmul(ps1[:, :D], mpT[:, kc, :],
                                         W("Wp1bbT", kc), start=False,
                                         stop=kc == 1)
                    hv = epool.tile([128, D], F32, tag="cbv")
                    nc.vector.scalar_tensor_tensor(hv[:], ps1[:, :D], 1.0,
                                                   V("bp1b"), ALU.mult, ALU.add)
                    lno = epool.tile([128, D], F32, tag="sb")
                    ln_epilogue(hv, V("plngb"), V("plnbb"), lno[:])
                    hbr = epool.tile([128, D], BF, tag="relu_bf")
                    nc.scalar.activation(hbr[:], lno[:], AF.Relu)
                    hbT = epool.tile([128, 2, 128], BF, tag="mpT")
                    for kc in range(2):
                        pst = trZ.tile([128, 128], BF, tag="tr")
                        nc.tensor.transpose(pst[:], hbr[:, ts(kc, 128)], ident[:])
                        nc.vector.tensor_copy(hbT[:, kc, :], pst[:])
                    ps2 = mmP.tile([128, 2 * D], F32, tag="mmp")
                    for kc in range(2):
                        nc.tensor.matmul(ps2[:, :128], hbT[:, kc, :],
                                         w128[:, w128i["Wp2bT"] + kc, :],
                                         start=kc == 0, stop=kc == 1)
                    hv2 = epool.tile([128, 128], F32, tag="h2")
                    nc.vector.scalar_tensor_tensor(hv2[:], ps2[:, :128], 1.0,
                                                   vb128[:, v128i["bp2b"], :],
                                                   ALU.mult, ALU.add)
                    rec = l2_recip(hv2, width=128)
                    hn = epool.tile([128, 128], BF, tag="h2n")
                    nc.scalar.activation(hn[:], hv2[:], AF.Copy, scale=rec[:])
                    pst = trZ.tile([128, 128], BF, tag="tr")
                    nc.tensor.transpose(pst[:], hn[:], ident[:])
                    hbnT = epool.tile([128, 128], BF, tag="hbnT")
                    nc.vector.tensor_copy(hbnT[:], pst[:])
                    for s in range(4):
                        w = SIMW[s]
                        pssim = mmP.tile([128, 2 * D], F32, tag="mmp")
                        nc.tensor.matmul(pssim[:, :w], hbnT[:],
                                         htn_T[:, ds(s * 512, w)],
                                         start=True, stop=True)
                        ob = opool.tile([128, 512], F16, tag="simout")
                        if s % 2 == 0:
                            nc.vector.tensor_copy(ob[:, :w], pssim[:, :w])
                        else:
                            nc.scalar.copy(ob[:, :w], pssim[:, :w])
                        nc.sync.dma_start(sim_d[nt][:, ds(s * 512, w)],
                                          ob[:, :w])

    nc.compile()
    return nc


# ---------------------------------------------------------------------------
# Entry point
# ---------------------------------------------------------------------------

def kernel(**inputs):
    in_maps, meta = _prep(inputs)
    nc = build_program(meta)
    res = run_bass_kernel_spmd(nc, in_maps, core_ids=list(range(NC)))
    sim = np.empty((N_B, N_T), np.float32)
    for c in range(NC):
        shard = np.asarray(res.results[c]["simO"]).reshape(BP, TP)
        sim[c * B_SH:(c + 1) * B_SH] = shard[:B_SH, :N_T].astype(np.float32)
    if meta["simb"] != 0.0:
        sim += np.float32(meta["simb"])
    return sim
